# Initial kernel scaffold
#
"""FlexibleThresholdedLoss on 8 Trainium2 NeuronCores.

Strategy (pure data parallel over the batch dim):
  - Each core gets 4 of the 32 images of both inputs, viewed as [128, 24576] f32.
  - Phase A (streaming from HBM, DMA-bound):
      d = a - b (DVE); u = |d| fp16 and q = d^2 fp16 resident in SBUF (ACT);
      global sums of u and q via PE ones-matmuls accumulating in PSUM.
  - One AllReduce of the two scalar partials -> global mae/mse means (t, s).
  - Phase B (on SBUF-resident u and q, no HBM traffic), in-domain max algebra:
      c1 = #[u >= t],  Sm1 = sum max(u, t)   (mae side)
      c2 = #[q >= s],  Sm2 = sum max(q, s)   (mse side, squared domain)
    masks/max tiles via DVE tensor_scalar (4x fp16), sums via PE ones-matmuls.
  - Host closes the algebra exactly:
      s1 = Sm1 - t*(N - c1)     (masked |d| sum;  mae_thr = s1/c1)
      s2 = Sm2 - s*(N - c2)     (masked d^2 sum;  mse_thr = s2/c2)
"""

import numpy as np

import concourse.bacc as bacc
import concourse.mybir as mybir
from concourse import bass_isa
from concourse.bass_utils import run_bass_kernel_spmd
from concourse.tile import TileContext

P = 128
FD = 24576            # per-partition elements per input tensor (per core)
N_CORES = 8
N_TOTAL = 32 * 3 * 512 * 512   # 25_165_824 global element count
CHUNK_A = 1024        # phase A chunk (24 chunks, 1 MB DMA each, 3 per HWDGE queue)
CHUNK_B = 4096        # phase B chunk (6 chunks)
MM_N = 512            # PE ones-matmul moving free dim (one PSUM bank)

F32 = mybir.dt.float32
F16 = mybir.dt.float16
ALU = mybir.AluOpType
ACTF = mybir.ActivationFunctionType
AX = mybir.AxisListType

N_A = FD // CHUNK_A   # 12
N_B = FD // CHUNK_B   # 6

# ablation controls for the timing loop (None = full)
LOOP_PARTS_A = None
LOOP_PARTS_B = None

PARTS_A = frozenset({"dma", "sub", "abs", "square", "pe_u", "pe_q"})
PARTS_B = frozenset({"m1", "c1", "m2", "c2"})


def _pe_sum(nc, ones, psum, src, start, stop=False):
    """Accumulate per-column partition sums of src [P, W] into psum [1, MM_N]."""
    w = src.shape[-1]
    for j in range(0, w, MM_N):
        nc.tensor.matmul(
            psum[:, 0:MM_N],
            ones[:, 0:1],
            src[:, j : j + MM_N],
            start=(start and j == 0),
            stop=(stop and j + MM_N >= w),
        )


def _emit_phase_a(nc, ones, ab_d, u, q, psum_u, psum_q, stage_pool, d_pool,
                  parts=PARTS_A, first=True):
    for c in range(N_A):
        sl = slice(c * CHUNK_A, (c + 1) * CHUNK_A)
        if "dma" not in parts:
            continue
        abt = stage_pool.tile([P, 2 * CHUNK_A], F32, tag="stage")
        nc.sync.dma_start(
            abt[:], ab_d[:, 2 * c * CHUNK_A : 2 * (c + 1) * CHUNK_A]
        )
        if "sub" not in parts:
            continue
        dt_ = d_pool.tile([P, CHUNK_A], F32, tag="d")
        nc.vector.tensor_tensor(
            dt_[:], abt[:, 0:CHUNK_A], abt[:, CHUNK_A : 2 * CHUNK_A],
            op=ALU.subtract,
        )
        if "abs" in parts:
            nc.scalar.activation(u[:, sl], dt_[:], ACTF.Abs)
        if "square" in parts:
            nc.scalar.activation(q[:, sl], dt_[:], ACTF.Square)
        if "pe_u" in parts:
            _pe_sum(nc, ones, psum_u, u[:, sl], start=(first and c == 0),
                    stop=(c == N_A - 1))
        if "pe_q" in parts:
            _pe_sum(nc, ones, psum_q, q[:, sl], start=(first and c == 0),
                    stop=(c == N_A - 1))


def _emit_phase_b(nc, ones, u, q, thr, psums, stage_pool, parts=PARTS_B,
                  first=True):
    t_ap = thr[:, 0:1]
    s_ap = thr[:, 1:2]
    specs = [
        ("m1", u, t_ap, ALU.max, "pm1"),
        ("c1", u, t_ap, ALU.is_ge, "pc1"),
        ("m2", q, s_ap, ALU.max, "pm2"),
        ("c2", q, s_ap, ALU.is_ge, "pc2"),
    ]
    for k in range(N_B):
        sl = slice(k * CHUNK_B, (k + 1) * CHUNK_B)
        for name, buf, scal, op, pname in specs:
            if name not in parts:
                continue
            ot = stage_pool.tile([P, CHUNK_B], F16, tag="stage")
            nc.vector.tensor_scalar(ot[:], buf[:, sl], scal, None, op0=op)
            _pe_sum(nc, ones, psums[pname], ot[:], start=(first and k == 0),
                    stop=(k == N_B - 1))


def _build_program(stop_after="full", loop_n=0, loop_cc=False):
    nc = bacc.Bacc("TRN2", target_bir_lowering=False)

    # host packs a and b interleaved per chunk: [P, n_chunks, 2, CHUNK_A]
    ab_d = nc.declare_dram_parameter("ab", [P, 2 * FD], F32, isOutput=False)
    out_d = nc.declare_dram_parameter("partials", [1, 16], F32, isOutput=True)

    with TileContext(nc) as tc:
        with (
            tc.tile_pool(name="ubuf", bufs=1) as ubuf_pool,
            tc.tile_pool(name="stage", bufs=8) as stage_pool,
            tc.tile_pool(name="dbuf", bufs=3) as d_pool,
            tc.tile_pool(name="small", bufs=1) as small_pool,
            tc.tile_pool(name="psum", bufs=1, space="PSUM") as psum_pool,
            tc.tile_pool(name="dram", bufs=1, space="DRAM") as dram_pool,
        ):
            u = ubuf_pool.tile([P, FD], F16, tag="u")
            q = ubuf_pool.tile([P, FD], F16, tag="q")

            ones = small_pool.tile([P, 1], F16, tag="ones")
            nc.vector.memset(ones[:], 1.0)

            psum_u = psum_pool.tile([1, MM_N], F32, tag="pu")
            psum_q = psum_pool.tile([1, MM_N], F32, tag="pq")
            psums = {
                n: psum_pool.tile([1, MM_N], F32, tag=n, name=n)
                for n in ("pm1", "pc1", "pm2", "pc2")
            }

            _emit_phase_a(
                nc, ones, ab_d, u, q, psum_u, psum_q, stage_pool, d_pool
            )

            # ---- core-local scalars, all-reduce across cores ----
            sums2 = small_pool.tile([1, 2], F32, tag="sums2")
            nc.vector.tensor_reduce(
                sums2[:, 0:1], psum_u[:, :], axis=AX.X, op=ALU.add
            )
            nc.vector.tensor_reduce(
                sums2[:, 1:2], psum_q[:, :], axis=AX.X, op=ALU.add
            )

            cc_in = dram_pool.tile([1, 2], F32, tag="cc_in")
            cc_out = dram_pool.tile([1, 2], F32, tag="cc_out", addr_space="Shared")
            nc.sync.dma_start(cc_in[:], sums2[:])
            nc.gpsimd.collective_compute(
                "AllReduce",
                ALU.add,
                replica_groups=[list(range(N_CORES))],
                ins=[cc_in.opt()],
                outs=[cc_out.opt()],
            )
            g = small_pool.tile([1, 2], F32, tag="g")
            nc.sync.dma_start(g[:], cc_out[:])

            # thresholds: t = mae mean, s = mse mean (both on partition 0)
            ts2 = small_pool.tile([1, 2], F32, tag="ts2")
            inv_n = 1.0 / float(N_TOTAL)
            nc.scalar.mul(ts2[:, 0:2], g[:, 0:2], inv_n)
            thr = small_pool.tile([P, 2], F32, tag="thr")
            nc.gpsimd.partition_broadcast(thr[:], ts2[:], channels=P)

            if stop_after == "thresholds":
                dbg = small_pool.tile([1, 16], F32, tag="dbg")
                nc.vector.memset(dbg[:], 0.0)
                nc.scalar.copy(dbg[:, 0:2], g[:, 0:2])
                nc.scalar.copy(dbg[:, 2:4], ts2[:, 0:2])
                nc.sync.dma_start(out_d[:], dbg[:])
            else:
                _emit_phase_b(nc, ones, u, q, thr, psums, stage_pool)

                # ---- final reductions + output row ----
                outrow = small_pool.tile([1, 16], F32, tag="outrow")
                nc.vector.memset(outrow[:], 0.0)
                nc.scalar.copy(outrow[:, 0:2], g[:, 0:2])     # G_u, G_q
                nc.scalar.copy(outrow[:, 2:4], ts2[:, 0:2])   # t, s
                for j, pname in enumerate(("pc1", "pm1", "pc2", "pm2")):
                    nc.vector.tensor_reduce(
                        outrow[:, 4 + j : 5 + j], psums[pname][:, :],
                        axis=AX.X, op=ALU.add,
                    )
                nc.sync.dma_start(out_d[:], outrow[:])

                if loop_n:
                    # timing rig: repeat the A+B workload loop_n more times
                    pa = LOOP_PARTS_A if LOOP_PARTS_A is not None else PARTS_A
                    pb = LOOP_PARTS_B if LOOP_PARTS_B is not None else PARTS_B
                    if loop_cc:
                        cc_in2 = dram_pool.tile([1, 2], F32, tag="cc_in2",
                                                name="cc_in2")
                        cc_out2 = dram_pool.tile([1, 2], F32, tag="cc_out2",
                                                 name="cc_out2",
                                                 addr_space="Shared")
                    with tc.For_i(0, loop_n, 1):
                        _emit_phase_a(
                            nc, ones, ab_d, u, q, psum_u, psum_q, stage_pool,
                            d_pool, parts=pa, first=True,
                        )
                        if loop_cc:
                            nc.vector.tensor_reduce(
                                sums2[:, 0:1], psum_u[:, :], axis=AX.X,
                                op=ALU.add,
                            )
                            nc.vector.tensor_reduce(
                                sums2[:, 1:2], psum_q[:, :], axis=AX.X,
                                op=ALU.add,
                            )
                            nc.sync.dma_start(cc_in2[:], sums2[:])
                            nc.gpsimd.collective_compute(
                                "AllReduce",
                                ALU.add,
                                replica_groups=[list(range(N_CORES))],
                                ins=[cc_in2.opt()],
                                outs=[cc_out2.opt()],
                            )
                            nc.sync.dma_start(g[:], cc_out2[:])
                            nc.scalar.mul(ts2[:, 0:2], g[:, 0:2], inv_n)
                            nc.gpsimd.partition_broadcast(
                                thr[:], ts2[:], channels=P
                            )
                        if pb:
                            _emit_phase_b(
                                nc, ones, u, q, thr, psums, stage_pool,
                                parts=pb, first=True,
                            )

    nc.compile()
    return nc


_NC_CACHE = None


def _get_program():
    global _NC_CACHE
    if _NC_CACHE is None:
        _NC_CACHE = _build_program()
    return _NC_CACHE


def _shard_inputs(input_img: np.ndarray, target_img: np.ndarray):
    a = np.asarray(input_img, dtype=np.float32)
    b = np.asarray(target_img, dtype=np.float32)
    per = a.shape[0] // N_CORES
    in_maps = []
    for i in range(N_CORES):
        sl = slice(i * per, (i + 1) * per)
        ai = np.ascontiguousarray(a[sl]).reshape(P, N_A, 1, CHUNK_A)
        bi = np.ascontiguousarray(b[sl]).reshape(P, N_A, 1, CHUNK_A)
        # interleave per chunk: [P, n_a, 2, CHUNK_A] -> [P, 2*FD]
        ab = np.concatenate([ai, bi], axis=2).reshape(P, 2 * FD)
        in_maps.append({"ab": np.ascontiguousarray(ab)})
    return in_maps


def _combine(results) -> np.float32:
    # identical on every core: global sums + thresholds
    row0 = results[0]["partials"].reshape(-1).astype(np.float64)
    g_u, g_q, t, s = row0[0], row0[1], row0[2], row0[3]
    # per-core masked partials: sum over cores
    c1 = sm1 = c2 = sm2 = 0.0
    for res in results:
        row = res["partials"].reshape(-1).astype(np.float64)
        c1 += row[4]
        sm1 += row[5]
        c2 += row[6]
        sm2 += row[7]

    n = float(N_TOTAL)
    mae_loss = g_u / n
    mse_loss = g_q / n

    s1 = sm1 - t * (n - c1)   # sum u over u >= t
    s2 = sm2 - s * (n - c2)   # sum q over q >= s

    mae_thr = s1 / c1 if c1 > 0 else 0.0
    mse_thr = s2 / c2 if c2 > 0 else 0.0

    combined_thr = 0.5 * mae_thr + 0.5 * mse_thr
    combined_non = 0.5 * mae_loss + 0.5 * mse_loss
    total = 0.5 * combined_thr + 0.5 * combined_non
    return np.float32(total)


def kernel(input_img: np.ndarray, target_img: np.ndarray) -> np.ndarray:
    import time as _time

    nc = _get_program()
    in_maps = _shard_inputs(input_img, target_img)
    last_err = None
    for attempt in range(3):
        try:
            res = run_bass_kernel_spmd(nc, in_maps, list(range(N_CORES)))
            return np.asarray(_combine(res.results))
        except Exception as e:  # transient device-unrecoverable states
            last_err = e
            _time.sleep(20 * (attempt + 1))
    raise last_err



# revision 1
# speedup vs baseline: 1.5254x; 1.5254x over previous
"""FlexibleThresholdedLoss on 8 Trainium2 NeuronCores.

Strategy (pure data parallel over the batch dim):
  - Each core gets 4 of the 32 images of both inputs, viewed as [128, 24576] f32.
  - Phase A (streaming from HBM, DMA-bound):
      d = a - b (DVE); u = |d| fp16 and q = d^2 fp16 resident in SBUF (ACT);
      global sums of u and q via PE ones-matmuls accumulating in PSUM.
  - One AllReduce of the two scalar partials -> global mae/mse means (t, s).
  - Phase B (on SBUF-resident u and q, no HBM traffic), in-domain max algebra:
      c1 = #[u >= t],  Sm1 = sum max(u, t)   (mae side)
      c2 = #[q >= s],  Sm2 = sum max(q, s)   (mse side, squared domain)
    masks/max tiles via DVE tensor_scalar (4x fp16), sums via PE ones-matmuls.
  - Host closes the algebra exactly:
      s1 = Sm1 - t*(N - c1)     (masked |d| sum;  mae_thr = s1/c1)
      s2 = Sm2 - s*(N - c2)     (masked d^2 sum;  mse_thr = s2/c2)
"""

import numpy as np

import concourse.bacc as bacc
import concourse.mybir as mybir
from concourse import bass_isa
from concourse.bass_utils import run_bass_kernel_spmd
from concourse.tile import TileContext

P = 128
FD = 24576            # per-partition elements per input tensor (per core)
N_CORES = 8
N_TOTAL = 32 * 3 * 512 * 512   # 25_165_824 global element count
CHUNK_A = 1024        # phase A chunk (24 chunks, 1 MB DMA each, 3 per HWDGE queue)
CHUNK_B = 4096        # phase B chunk (6 chunks)
MM_N = 512            # PE ones-matmul moving free dim (one PSUM bank)

F32 = mybir.dt.float32
F16 = mybir.dt.float16
ALU = mybir.AluOpType
ACTF = mybir.ActivationFunctionType
AX = mybir.AxisListType

N_A = FD // CHUNK_A   # 12
N_B = FD // CHUNK_B   # 6

# ablation controls for the timing loop (None = full)
LOOP_PARTS_A = None
LOOP_PARTS_B = None

PARTS_A = frozenset({"dma", "sub", "abs", "square", "pe_u", "pe_q"})
PARTS_B = frozenset({"m1", "c1", "m2", "c2"})


def _pe_sum(nc, ones, psum, src, start, stop=False):
    """Accumulate per-column partition sums of src [P, W] into psum [1, MM_N]."""
    w = src.shape[-1]
    for j in range(0, w, MM_N):
        nc.tensor.matmul(
            psum[:, 0:MM_N],
            ones[:, 0:1],
            src[:, j : j + MM_N],
            start=(start and j == 0),
            stop=(stop and j + MM_N >= w),
        )


def _emit_phase_a(nc, ones, ab_d, u, q, psum_u, psum_q, stage_pool, d_pool,
                  parts=PARTS_A, first=True):
    for c in range(N_A):
        sl = slice(c * CHUNK_A, (c + 1) * CHUNK_A)
        if "dma" not in parts:
            continue
        abt = stage_pool.tile([P, 2 * CHUNK_A], F32, tag="stage")
        nc.sync.dma_start(
            abt[:], ab_d[:, 2 * c * CHUNK_A : 2 * (c + 1) * CHUNK_A]
        )
        if "sub" not in parts:
            continue
        dt_ = d_pool.tile([P, CHUNK_A], F32, tag="d")
        nc.vector.tensor_tensor(
            dt_[:], abt[:, 0:CHUNK_A], abt[:, CHUNK_A : 2 * CHUNK_A],
            op=ALU.subtract,
        )
        if "abs" in parts:
            nc.scalar.activation(u[:, sl], dt_[:], ACTF.Abs)
        if "square" in parts:
            nc.scalar.activation(q[:, sl], dt_[:], ACTF.Square)
        if "pe_u" in parts:
            _pe_sum(nc, ones, psum_u, u[:, sl], start=(first and c == 0),
                    stop=(c == N_A - 1))
        if "pe_q" in parts:
            _pe_sum(nc, ones, psum_q, q[:, sl], start=(first and c == 0),
                    stop=(c == N_A - 1))


def _emit_phase_b(nc, ones, u, q, thr, psums, stage_pool, parts=PARTS_B,
                  first=True):
    t_ap = thr[:, 0:1]
    s_ap = thr[:, 1:2]
    specs = [
        ("m1", u, t_ap, ALU.max, "pm1"),
        ("c1", u, t_ap, ALU.is_ge, "pc1"),
        ("m2", q, s_ap, ALU.max, "pm2"),
        ("c2", q, s_ap, ALU.is_ge, "pc2"),
    ]
    for k in range(N_B):
        sl = slice(k * CHUNK_B, (k + 1) * CHUNK_B)
        for name, buf, scal, op, pname in specs:
            if name not in parts:
                continue
            ot = stage_pool.tile([P, CHUNK_B], F16, tag="stage")
            nc.vector.tensor_scalar(ot[:], buf[:, sl], scal, None, op0=op)
            _pe_sum(nc, ones, psums[pname], ot[:], start=(first and k == 0),
                    stop=(k == N_B - 1))


def _build_program(stop_after="full", loop_n=0, loop_cc=False):
    nc = bacc.Bacc("TRN2", target_bir_lowering=False)

    # host packs a and b interleaved per chunk: [P, n_chunks, 2, CHUNK_A]
    ab_d = nc.declare_dram_parameter("ab", [P, 2 * FD], F32, isOutput=False)
    out_d = nc.declare_dram_parameter("partials", [1, 16], F32, isOutput=True)

    with TileContext(nc) as tc:
        with (
            tc.tile_pool(name="ubuf", bufs=1) as ubuf_pool,
            tc.tile_pool(name="stage", bufs=8) as stage_pool,
            tc.tile_pool(name="dbuf", bufs=3) as d_pool,
            tc.tile_pool(name="small", bufs=1) as small_pool,
            tc.tile_pool(name="psum", bufs=1, space="PSUM") as psum_pool,
            tc.tile_pool(name="dram", bufs=1, space="DRAM") as dram_pool,
        ):
            u = ubuf_pool.tile([P, FD], F16, tag="u")
            q = ubuf_pool.tile([P, FD], F16, tag="q")

            ones = small_pool.tile([P, 1], F16, tag="ones")
            nc.vector.memset(ones[:], 1.0)

            psum_u = psum_pool.tile([1, MM_N], F32, tag="pu")
            psum_q = psum_pool.tile([1, MM_N], F32, tag="pq")
            psums = {
                n: psum_pool.tile([1, MM_N], F32, tag=n, name=n)
                for n in ("pm1", "pc1", "pm2", "pc2")
            }

            _emit_phase_a(
                nc, ones, ab_d, u, q, psum_u, psum_q, stage_pool, d_pool
            )

            # ---- core-local scalars, all-reduce across cores ----
            sums2 = small_pool.tile([1, 2], F32, tag="sums2")
            nc.vector.tensor_reduce(
                sums2[:, 0:1], psum_u[:, :], axis=AX.X, op=ALU.add
            )
            nc.vector.tensor_reduce(
                sums2[:, 1:2], psum_q[:, :], axis=AX.X, op=ALU.add
            )

            cc_in = dram_pool.tile([1, 2], F32, tag="cc_in")
            cc_out = dram_pool.tile([1, 2], F32, tag="cc_out", addr_space="Shared")
            nc.sync.dma_start(cc_in[:], sums2[:])
            nc.gpsimd.collective_compute(
                "AllReduce",
                ALU.add,
                replica_groups=[list(range(N_CORES))],
                ins=[cc_in.opt()],
                outs=[cc_out.opt()],
            )
            g = small_pool.tile([1, 2], F32, tag="g")
            nc.sync.dma_start(g[:], cc_out[:])

            # thresholds: t = mae mean, s = mse mean (both on partition 0)
            ts2 = small_pool.tile([1, 2], F32, tag="ts2")
            inv_n = 1.0 / float(N_TOTAL)
            nc.scalar.mul(ts2[:, 0:2], g[:, 0:2], inv_n)
            thr = small_pool.tile([P, 2], F32, tag="thr")
            nc.gpsimd.partition_broadcast(thr[:], ts2[:], channels=P)

            if stop_after == "thresholds":
                dbg = small_pool.tile([1, 16], F32, tag="dbg")
                nc.vector.memset(dbg[:], 0.0)
                nc.scalar.copy(dbg[:, 0:2], g[:, 0:2])
                nc.scalar.copy(dbg[:, 2:4], ts2[:, 0:2])
                nc.sync.dma_start(out_d[:], dbg[:])
            else:
                _emit_phase_b(nc, ones, u, q, thr, psums, stage_pool)

                # ---- final reductions + output row ----
                outrow = small_pool.tile([1, 16], F32, tag="outrow")
                nc.vector.memset(outrow[:], 0.0)
                nc.scalar.copy(outrow[:, 0:2], g[:, 0:2])     # G_u, G_q
                nc.scalar.copy(outrow[:, 2:4], ts2[:, 0:2])   # t, s
                for j, pname in enumerate(("pc1", "pm1", "pc2", "pm2")):
                    nc.vector.tensor_reduce(
                        outrow[:, 4 + j : 5 + j], psums[pname][:, :],
                        axis=AX.X, op=ALU.add,
                    )
                nc.sync.dma_start(out_d[:], outrow[:])

                if loop_n:
                    # timing rig: repeat the A+B workload loop_n more times
                    pa = LOOP_PARTS_A if LOOP_PARTS_A is not None else PARTS_A
                    pb = LOOP_PARTS_B if LOOP_PARTS_B is not None else PARTS_B
                    if loop_cc:
                        cc_in2 = dram_pool.tile([1, 2], F32, tag="cc_in2",
                                                name="cc_in2")
                        cc_out2 = dram_pool.tile([1, 2], F32, tag="cc_out2",
                                                 name="cc_out2",
                                                 addr_space="Shared")
                    with tc.For_i(0, loop_n, 1):
                        _emit_phase_a(
                            nc, ones, ab_d, u, q, psum_u, psum_q, stage_pool,
                            d_pool, parts=pa, first=True,
                        )
                        if loop_cc:
                            nc.vector.tensor_reduce(
                                sums2[:, 0:1], psum_u[:, :], axis=AX.X,
                                op=ALU.add,
                            )
                            nc.vector.tensor_reduce(
                                sums2[:, 1:2], psum_q[:, :], axis=AX.X,
                                op=ALU.add,
                            )
                            nc.sync.dma_start(cc_in2[:], sums2[:])
                            nc.gpsimd.collective_compute(
                                "AllReduce",
                                ALU.add,
                                replica_groups=[list(range(N_CORES))],
                                ins=[cc_in2.opt()],
                                outs=[cc_out2.opt()],
                            )
                            nc.sync.dma_start(g[:], cc_out2[:])
                            nc.scalar.mul(ts2[:, 0:2], g[:, 0:2], inv_n)
                            nc.gpsimd.partition_broadcast(
                                thr[:], ts2[:], channels=P
                            )
                        if pb:
                            _emit_phase_b(
                                nc, ones, u, q, thr, psums, stage_pool,
                                parts=pb, first=True,
                            )

    nc.compile()
    return nc


_NC_CACHE = None


def _get_program():
    global _NC_CACHE
    if _NC_CACHE is None:
        _NC_CACHE = _build_program()
    return _NC_CACHE


def _shard_inputs(input_img: np.ndarray, target_img: np.ndarray):
    a = np.asarray(input_img, dtype=np.float32)
    b = np.asarray(target_img, dtype=np.float32)
    per = a.shape[0] // N_CORES
    in_maps = []
    for i in range(N_CORES):
        sl = slice(i * per, (i + 1) * per)
        ai = np.ascontiguousarray(a[sl]).reshape(P, N_A, 1, CHUNK_A)
        bi = np.ascontiguousarray(b[sl]).reshape(P, N_A, 1, CHUNK_A)
        # interleave per chunk: [P, n_a, 2, CHUNK_A] -> [P, 2*FD]
        ab = np.concatenate([ai, bi], axis=2).reshape(P, 2 * FD)
        in_maps.append({"ab": np.ascontiguousarray(ab)})
    return in_maps


def _combine(results) -> np.float32:
    # identical on every core: global sums + thresholds
    row0 = results[0]["partials"].reshape(-1).astype(np.float64)
    g_u, g_q, t, s = row0[0], row0[1], row0[2], row0[3]
    # per-core masked partials: sum over cores
    c1 = sm1 = c2 = sm2 = 0.0
    for res in results:
        row = res["partials"].reshape(-1).astype(np.float64)
        c1 += row[4]
        sm1 += row[5]
        c2 += row[6]
        sm2 += row[7]

    n = float(N_TOTAL)
    mae_loss = g_u / n
    mse_loss = g_q / n

    s1 = sm1 - t * (n - c1)   # sum u over u >= t
    s2 = sm2 - s * (n - c2)   # sum q over q >= s

    mae_thr = s1 / c1 if c1 > 0 else 0.0
    mse_thr = s2 / c2 if c2 > 0 else 0.0

    combined_thr = 0.5 * mae_thr + 0.5 * mse_thr
    combined_non = 0.5 * mae_loss + 0.5 * mse_loss
    total = 0.5 * combined_thr + 0.5 * combined_non
    return np.float32(total)


def kernel(input_img: np.ndarray, target_img: np.ndarray) -> np.ndarray:
    import time as _time

    nc = _get_program()
    in_maps = _shard_inputs(input_img, target_img)
    last_err = None
    for attempt in range(3):
        try:
            res = run_bass_kernel_spmd(nc, in_maps, list(range(N_CORES)))
            return np.asarray(_combine(res.results))
        except Exception as e:  # transient device-unrecoverable states
            last_err = e
            _time.sleep(20 * (attempt + 1))
    raise last_err

